# revision 9
# baseline (speedup 1.0000x reference)
"""Weighted 2D Kabsch (umeyama-style) kernel for Trainium2, 8 NeuronCores.

Math: the reference pads 2D coords with z=0, so the 3x3 SVD collapses to a
2x2 problem.  For the 2x2 weighted cross-covariance A = sum_n w (t - tbar)
(s - sbar)^T, the rotation block of R = U diag(1,1,det) V^T is exactly the
orthogonal polar factor of A, which has a closed form:

    s   = sign(det A)
    x   = A00 + s*A11 ;  y = A10 - s*A01 ;  r = sqrt(x^2 + y^2)
    Q   = [[x, -s*y], [y, s*x]] / r          (2x2 block of R)
    R   = [[Q, 0], [0, s]]
    t   = tgt_centroid - Q @ src_centroid    (z component exactly 0)

Device work: per-batch weighted reductions over N=4096 (the memory-bound
part) + tiny elementwise finalization.  Pure data-parallel: batch 1024 is
split 128-per-core across 8 cores; each core maps its 128 batches onto the
128 SBUF partitions and streams N in chunks.
"""

import numpy as np

import concourse.bacc as bacc
import concourse.bass as bass
import concourse.tile as tile
from concourse import mybir
from concourse.bass_utils import run_bass_kernel_spmd

B, N = 1024, 4096
NCORES = 8
BLOC = B // NCORES  # 128 batches per core == SBUF partition count
EPS = 1e-4

NC_CHUNK = 1024
NCHUNKS = N // NC_CHUNK

F32 = mybir.dt.float32
ALU = mybir.AluOpType
ACTF = mybir.ActivationFunctionType


def _emit(tc: tile.TileContext, src, tgt, w, r_out, t_out, ctx, reps=1):
    nc = tc.nc
    P = BLOC

    data = ctx.enter_context(tc.tile_pool(name="data", bufs=3))
    prod = ctx.enter_context(tc.tile_pool(name="prod", bufs=2))
    scr = ctx.enter_context(tc.tile_pool(name="scr", bufs=2))
    singles = ctx.enter_context(tc.tile_pool(name="singles", bufs=1))

    # per-chunk partial sums, reduced at the end
    part = {
        k: singles.tile([P, NCHUNKS], F32, tag=f"part_{k}", name=f"part_{k}")
        for k in ("sw", "msx", "msy", "mtx", "mty", "m00", "m01", "m10", "m11")
    }
    st = {
        k: singles.tile([P, 1], F32, tag=f"st_{k}", name=f"st_{k}") for k in part
    }
    _tmps = {}

    def tmp(nm):
        if nm not in _tmps:
            _tmps[nm] = singles.tile([P, 1], F32, tag=f"tmp_{nm}", name=f"tmp_{nm}")
        return _tmps[nm]

    r_t = singles.tile([P, 9], F32, tag="r_t")
    t_t2 = singles.tile([P, 3], F32, tag="t_t2")

    def one_pass():
        for c in range(NCHUNKS):
            lo, hi = c * NC_CHUNK, (c + 1) * NC_CHUNK
            s_t = data.tile([P, NC_CHUNK, 2], F32, tag="s", name="s_t")
            t_t = data.tile([P, NC_CHUNK, 2], F32, tag="t", name="t_t")
            w_t = data.tile([P, NC_CHUNK], F32, tag="w", name="w_t")
            nc.sync.dma_start(out=s_t, in_=src[:, lo:hi, :])
            nc.sync.dma_start(out=t_t, in_=tgt[:, lo:hi, :])
            nc.sync.dma_start(out=w_t, in_=w[:, lo:hi])

            sx, sy = s_t[:, :, 0], s_t[:, :, 1]
            tx, ty = t_t[:, :, 0], t_t[:, :, 1]

            p_t = prod.tile([P, NC_CHUNK], F32, tag="p", name="p_t")  # w * tx
            q_t = prod.tile([P, NC_CHUNK], F32, tag="q", name="q_t")  # w * ty
            dve_scr = scr.tile([P, NC_CHUNK], F32, tag="dve_scr", name="dve_scr")
            act_scr = scr.tile([P, NC_CHUNK], F32, tag="act_scr", name="act_scr")

            # gpsimd computes the weighted-target products (Pool has no
            # fused multiply+reduce opcode, plain TENSOR_TENSOR only)
            nc.gpsimd.tensor_tensor(out=p_t, in0=w_t, in1=tx, op=ALU.mult)
            nc.gpsimd.tensor_tensor(out=q_t, in0=w_t, in1=ty, op=ALU.mult)
            # vector engine: 6 fused multiply+reduce passes.
            # (tensor_tensor_reduce crashes the device in this environment;
            # scalar_tensor_tensor with accum_out is the working equivalent:
            # out = (in0 * 1.0) * in1, accum_out = sum(out).)
            for in0, in1, key in ((p_t, sx, "m00"), (p_t, sy, "m01"),
                                  (q_t, sx, "m10"), (q_t, sy, "m11"),
                                  (w_t, sx, "msx"), (w_t, sy, "msy")):
                nc.vector.scalar_tensor_tensor(
                    out=dve_scr, in0=in0, scalar=1.0, in1=in1,
                    op0=ALU.mult, op1=ALU.mult,
                    accum_out=part[key][:, c : c + 1])
            # scalar engine: plain sums of w, p, q via copy-with-accumulate
            nc.scalar.activation(
                out=act_scr, in_=w_t, func=ACTF.Copy,
                accum_out=part["sw"][:, c : c + 1])
            nc.scalar.activation(
                out=act_scr, in_=p_t, func=ACTF.Copy,
                accum_out=part["mtx"][:, c : c + 1])
            nc.scalar.activation(
                out=act_scr, in_=q_t, func=ACTF.Copy,
                accum_out=part["mty"][:, c : c + 1])

        # ---- finalize: all [P, 1] elementwise ----
        for k in part:
            nc.vector.tensor_reduce(
                out=st[k], in_=part[k], axis=mybir.AxisListType.X, op=ALU.add)

        v = nc.vector
        wbar = tmp("wbar")
        v.tensor_scalar_add(wbar, st["sw"], EPS)
        inv = tmp("inv")
        v.reciprocal(inv, wbar)
        csx, csy, tcx, tcy = tmp("csx"), tmp("csy"), tmp("tcx"), tmp("tcy")
        v.tensor_mul(csx, st["msx"], inv)
        v.tensor_mul(csy, st["msy"], inv)
        v.tensor_mul(tcx, st["mtx"], inv)
        v.tensor_mul(tcy, st["mty"], inv)

        # A_ij = M_ij - mt_i * cs_j
        a = {}
        for (i, mt_i) in (("0", st["mtx"]), ("1", st["mty"])):
            for (j, cs_j) in (("0", csx), ("1", csy)):
                u = tmp(f"u{i}{j}")
                v.tensor_mul(u, mt_i, cs_j)
                a[i + j] = tmp(f"a{i}{j}")
                v.tensor_sub(a[i + j], st[f"m{i}{j}"], u)

        d1, d2, d = tmp("d1"), tmp("d2"), tmp("d")
        v.tensor_mul(d1, a["00"], a["11"])
        v.tensor_mul(d2, a["01"], a["10"])
        v.tensor_sub(d, d1, d2)
        sgn_b, sgn = tmp("sgn_b"), tmp("sgn")
        v.tensor_scalar(sgn_b, d, 0.0, None, ALU.is_ge)            # 1.0 / 0.0
        v.tensor_scalar(sgn, sgn_b, 2.0, -1.0, ALU.mult, ALU.add)  # +1 / -1

        x = tmp("x")   # A00 + s*A11
        v.scalar_tensor_tensor(out=x, in0=a["11"], scalar=sgn, in1=a["00"],
                               op0=ALU.mult, op1=ALU.add)
        yn = tmp("yn")  # s*A01 - A10 == -y
        v.scalar_tensor_tensor(out=yn, in0=a["01"], scalar=sgn, in1=a["10"],
                               op0=ALU.mult, op1=ALU.subtract)
        y2, r2 = tmp("y2"), tmp("r2")
        v.tensor_mul(y2, yn, yn)
        v.scalar_tensor_tensor(out=r2, in0=x, scalar=x, in1=y2,
                               op0=ALU.mult, op1=ALU.add)
        r2inv, rinv = tmp("r2inv"), tmp("rinv")
        v.reciprocal(r2inv, r2)
        nc.scalar.activation(out=rinv, in_=r2inv, func=ACTF.Sqrt)

        v.memset(r_t, 0.0)
        v.memset(t_t2, 0.0)

        q00 = r_t[:, 0:1]; q01 = r_t[:, 1:2]
        q10 = r_t[:, 3:4]; q11 = r_t[:, 4:5]
        v.tensor_mul(q00, x, rinv)           # x/r
        ynr = tmp("ynr")
        v.tensor_mul(ynr, yn, rinv)          # -y/r
        v.tensor_mul(q01, sgn, ynr)          # -s*y/r
        v.tensor_scalar_mul(q10, ynr, -1.0)  # y/r
        v.tensor_mul(q11, sgn, q00)          # s*x/r
        nc.vector.tensor_copy(r_t[:, 8:9], sgn)

        # t2 = tc - Q @ cs
        u1, u2, inner = tmp("u1"), tmp("u2"), tmp("inner")
        v.tensor_mul(u1, q00, csx)
        v.tensor_mul(u2, q01, csy)
        v.tensor_add(inner, u1, u2)
        v.tensor_sub(t_t2[:, 0:1], tcx, inner)
        v.tensor_mul(u1, q10, csx)
        v.tensor_mul(u2, q11, csy)
        v.tensor_add(inner, u1, u2)
        v.tensor_sub(t_t2[:, 1:2], tcy, inner)

    # `reps` re-runs of the whole computation inside one NEFF let test.py
    # back out dispatch overhead from wall-clock deltas.  Each rep rewrites
    # (not accumulates) every partial/output, so results are rep-invariant.
    for _rep in range(reps):
        one_pass()

    nc.sync.dma_start(out=r_out, in_=r_t)
    nc.sync.dma_start(out=t_out, in_=t_t2)


_NC_CACHE = None


def _build(reps=1):
    global _NC_CACHE
    if _NC_CACHE is None:
        _NC_CACHE = {}
    if reps in _NC_CACHE:
        return _NC_CACHE[reps]
    from contextlib import ExitStack

    nc = bacc.Bacc()
    src = nc.declare_dram_parameter("src", [BLOC, N, 2], F32, isOutput=False)
    tgt = nc.declare_dram_parameter("tgt", [BLOC, N, 2], F32, isOutput=False)
    w = nc.declare_dram_parameter("w", [BLOC, N], F32, isOutput=False)
    r_out = nc.declare_dram_parameter("r_out", [BLOC, 9], F32, isOutput=True)
    t_out = nc.declare_dram_parameter("t_out", [BLOC, 3], F32, isOutput=True)
    with tile.TileContext(nc) as tc:
        with ExitStack() as ctx:
            _emit(tc, src[:], tgt[:], w[:], r_out[:], t_out[:], ctx, reps=reps)
    nc.finalize()
    _NC_CACHE[reps] = nc
    return nc


def _in_maps(src_coords, tgt_coords, weights):
    maps = []
    for c in range(NCORES):
        sl = slice(c * BLOC, (c + 1) * BLOC)
        maps.append({
            "src": np.ascontiguousarray(src_coords[sl], dtype=np.float32),
            "tgt": np.ascontiguousarray(tgt_coords[sl], dtype=np.float32),
            "w": np.ascontiguousarray(weights[sl, 0, :], dtype=np.float32),
        })
    return maps


def _assemble(results):
    R = np.concatenate([r["r_out"] for r in results], axis=0).reshape(B, 3, 3)
    t = np.concatenate([r["t_out"] for r in results], axis=0).reshape(B, 3, 1)
    return R.astype(np.float32), t.astype(np.float32)


def run(src_coords, tgt_coords, weights, reps=1, **kw):
    nc = _build(reps=reps)
    res = run_bass_kernel_spmd(
        nc, _in_maps(src_coords, tgt_coords, weights), list(range(NCORES)), **kw)
    return _assemble(res.results), res


def kernel(src_coords, tgt_coords, weights):
    (R, t), _ = run(src_coords, tgt_coords, weights)
    return R, t
